# revision 19
# baseline (speedup 1.0000x reference)
"""Trainium2 Bass kernel for DiagonalGMMPosterior (vq_codebook).

Reference computation (per batch b, descriptor n, cluster k):
    dist[k,n]  = sum_d (x[d,n] - mu_n[k,d])^2 * exp(-log_sigma[k,d])
    logits     = -dist + log_alpha[k] - 0.5 * sum_d log_sigma[k,d]
    out[k,n]   = softmax_k(logits)

Device strategy (8 NeuronCores, data-parallel over the batch axis):
  * Host folds all (K,D) parameter math into two GEMM weight matrices and
    a per-cluster constant, then CENTERS them across K (softmax is
    invariant to per-n shifts) so logits stay within ~±17 — no per-n max.
  * x is pre-converted to fp16 on host: halves input HBM traffic.  fp16
    GEMMs also unlock PE column tiling (tile_position), which fp32r does
    not support.
  * Two 2048-column halves A/B of each 4096-column group are STACKED on
    the 128 PSUM partitions: A's dist GEMMs write partitions 0:64 (PE
    tile (0,0)), B's write 64:128 (tile (0,64)).  Everything downstream
    processes both halves in one 128-partition pass, halving the column
    count of exp / ones-GEMM / reciprocal / multiply:
       pd   = W1^T @ x^2 + W2^T @ x   stacked       (TensorE, fp16)
       et   = exp(pd + cc)                          (ScalarE -> f32r)
       pd   = ones_blockdiag^T @ et                 (TensorE overwrites
              the spent pd banks: per-half partition sum + broadcast;
              a separate pb pool would not fit the 8 PSUM banks)
       r    = 1/pd                                  (VectorE, ~18-bit)
       ot   = et * r -> fp16                        (split DVE / Pool)
  * Large groups (4096 cols) halve the cross-engine semaphore count —
    sync latency, not engine throughput, paced the smaller-tile version.
  * Elementwise work is split across DVE/ScalarE/Pool by measured HW
    rates (DVE ~0.65-1.1 ns/col, Act ~0.95 ns/col, Pool ~2.4 ns/col with
    ~1.2us fixed): square on DVE+Act, multiply on DVE+Pool.
  * fp16 output in the stacked layout; host unstacks and widens.
"""

import numpy as np

import concourse.bacc as bacc
import concourse.bass as bass
import concourse.tile as tile
from concourse import mybir
from concourse.bass_utils import run_bass_kernel_spmd

B, D, N, K = 16, 128, 16384, 64
NCORES = 8
BPC = B // NCORES   # batches per core
PW = 2048           # columns per stacked half (4 PSUM banks)
GRP = 2 * PW        # n-columns consumed per group (halves A+B)
SQ_DVE = 1536       # square columns on DVE; rest on ScalarE
M_DVE = 1024        # multiply columns on DVE; rest on Pool

F32 = mybir.dt.float32
F32R = mybir.dt.float32r
F16 = mybir.dt.float16

_CACHE = {}


def _build_nc():
    # Bacc (not raw Bass): its compile() pass legalizes Tile's multi-wait
    # instructions down to the 1-wait-per-instruction hardware limit.
    nc = bacc.Bacc("TRN2", target_bir_lowering=False, debug=False)
    x_in = nc.declare_dram_parameter("x", [BPC, D, N], F16, isOutput=False)
    w1_in = nc.declare_dram_parameter("w1", [D, K], F16, isOutput=False)
    w2_in = nc.declare_dram_parameter("w2", [D, K], F16, isOutput=False)
    cc_in = nc.declare_dram_parameter("cc", [2 * K, 1], F32, isOutput=False)
    ones_in = nc.declare_dram_parameter("ones_bd", [2 * K, 2 * K], F32R, isOutput=False)
    # output stays in the stacked layout; the host unstacks
    out_ext = nc.declare_dram_parameter("out", [BPC, 2 * K, N // 2], F16, isOutput=True)

    with tile.TileContext(nc) as tc:
        with (
            tc.tile_pool(name="consts", bufs=1) as consts,
            tc.tile_pool(name="xp", bufs=2) as xp,
            tc.tile_pool(name="xqp", bufs=3) as xqp,
            tc.tile_pool(name="ep", bufs=3) as ep,
            tc.tile_pool(name="rp", bufs=3) as rp,
            tc.tile_pool(name="op", bufs=2) as op,
            tc.tile_pool(name="pd", bufs=2, space="PSUM") as pdp,
        ):
            w1_sb = consts.tile([D, K], F16)
            nc.sync.dma_start(out=w1_sb, in_=w1_in[:, :])
            w2_sb = consts.tile([D, K], F16)
            nc.sync.dma_start(out=w2_sb, in_=w2_in[:, :])
            cc_sb = consts.tile([2 * K, 1], F32)
            nc.sync.dma_start(out=cc_sb, in_=cc_in[:, :])
            ones_bd = consts.tile([2 * K, 2 * K], F32R)
            nc.sync.dma_start(out=ones_bd, in_=ones_in[:, :])

            n_grp = N // GRP  # 4 per batch row
            groups = [(b, g) for b in range(BPC) for g in range(n_grp)]
            NG = len(groups)
            st = [dict() for _ in range(NG)]

            # software-pipelined emission: each engine's in-order stream
            # interleaves stages of consecutive groups so no stage
            # head-of-line-blocks the next group's earlier stage
            def s0_load(i):
                # one DMA covers two groups (16KB per partition row):
                # halves the per-dispatch sequencer cost (~590ns each)
                if i % 2:
                    st[i]["xt"] = st[i - 1]["xt_big"][:, GRP:]
                    return
                b, g = groups[i]
                n0 = g * GRP
                xt = xp.tile([D, 2 * GRP], F16, tag="xt")
                nc.sync.dma_start(out=xt, in_=x_in[b, :, n0 : n0 + 2 * GRP])
                st[i]["xt_big"] = xt
                st[i]["xt"] = xt[:, :GRP]

            def s1_square(i):
                xt = st[i]["xt"]
                xsq = xqp.tile([D, GRP], F16, tag="xsq")
                nc.vector.tensor_mul(
                    xsq[:, :SQ_DVE], xt[:, :SQ_DVE], xt[:, :SQ_DVE]
                )
                nc.scalar.activation(
                    out=xsq[:, SQ_DVE:], in_=xt[:, SQ_DVE:],
                    func=mybir.ActivationFunctionType.Square,
                )
                st[i]["xsq"] = xsq

            def s2_dist(i):
                xt, xsq = st[i]["xt"], st[i]["xsq"]
                # halves A (cols 0:PW) and B (cols PW:2PW) stacked on the
                # 128 PSUM partitions via PE column tiling.  Each PSUM
                # bank holds 512 fp32 columns -> 4 chains per half.
                pd_t = pdp.tile([2 * K, PW], F32, tag="pd")
                for half, p0 in ((0, 0), (1, K)):
                    coff = half * PW
                    for h in range(PW // 512):
                        sl = slice(h * 512, (h + 1) * 512)
                        msl = slice(coff + h * 512, coff + (h + 1) * 512)
                        nc.tensor.matmul(
                            pd_t[p0 : p0 + K, sl], w1_sb[:, :], xsq[:, msl],
                            start=True, stop=False,
                        )
                        nc.tensor.matmul(
                            pd_t[p0 : p0 + K, sl], w2_sb[:, :], xt[:, msl],
                            start=False, stop=True,
                        )
                st[i]["pd"] = pd_t

            def s3_exp(i):
                pd_t = st[i]["pd"]
                # f32r: uniform-fp32 operands keep the DVE multiply on its
                # fast path (2-byte et forces a ~4x slower mixed-dtype
                # path) and stream the ones-GEMM at 1 cycle/row
                et = ep.tile([2 * K, PW], F32R, tag="et")
                nc.scalar.activation(
                    out=et, in_=pd_t,
                    func=mybir.ActivationFunctionType.Exp,
                    bias=cc_sb, scale=1.0,
                )
                st[i]["et"] = et
                st[i].pop("xt")
                st[i].pop("xsq")

            def s4_den(i):
                et = st[i]["et"]
                # block-diagonal ones: rows 0:64 sum half A, 64:128 sum
                # half B, each broadcast to its own partition range.  The
                # output overwrites the group's own spent pd banks.
                pd_t = st[i].pop("pd")
                for h in range(PW // 512):
                    sl = slice(h * 512, (h + 1) * 512)
                    nc.tensor.matmul(
                        pd_t[:, sl], ones_bd[:, :], et[:, sl],
                        start=True, stop=True,
                    )
                st[i]["pb"] = pd_t

            def s5_recip(i):
                pb_t = st[i].pop("pb")
                r_all = rp.tile([2 * K, PW], F32, tag="r")
                # ~18-bit-accurate custom-DVE reciprocal; the sum is
                # strictly positive and well inside fp32 normal range
                nc.vector.reciprocal_approx_fast(out=r_all, in_=pb_t)
                st[i]["r"] = r_all

            def s6_mult(i):
                et, r_all = st[i].pop("et"), st[i].pop("r")
                # double-wide output tile shared by group pairs: the store
                # then moves 8KB HBM rows in one dispatch per two groups
                if i % 2 == 0:
                    ot2 = op.tile([2 * K, 2 * PW], F16, tag="ot")
                    st[i]["ot2"] = ot2
                else:
                    ot2 = st[i - 1]["ot2"]
                c0 = (i % 2) * PW
                nc.vector.tensor_mul(
                    ot2[:, c0 : c0 + M_DVE],
                    et.bitcast(F32)[:, :M_DVE], r_all[:, :M_DVE],
                )
                nc.gpsimd.tensor_mul(
                    ot2[:, c0 + M_DVE : c0 + PW],
                    et.bitcast(F32)[:, M_DVE:], r_all[:, M_DVE:],
                )
                st[i]["ot"] = ot2

            def s7_store(i):
                if i % 2 == 0:
                    return
                b, g = groups[i]
                ot2 = st[i - 1].pop("ot2")
                st[i].pop("ot")
                st[i - 1].pop("ot")
                nc.sync.dma_start(
                    out=out_ext[b, :, (g - 1) * PW : (g + 1) * PW], in_=ot2[:, :]
                )

            stages = [
                s0_load, s1_square, s2_dist, s3_exp,
                s4_den, s5_recip, s6_mult, s7_store,
            ]
            NS = len(stages)
            # downstream stages emitted first within each tick so no
            # engine's in-order queue blocks a later group's earlier stage
            for tick in range(NG + NS - 1):
                for k in reversed(range(NS)):
                    i = tick - k
                    if 0 <= i < NG:
                        stages[k](i)
    nc.compile()
    return nc


def _host_params(mu, log_sigma, log_alpha):
    mu64 = mu.astype(np.float64)
    mu_n = mu64 / np.maximum(
        np.linalg.norm(mu64, axis=1, keepdims=True), 1e-12
    )
    sinv = np.exp(-log_sigma.astype(np.float64))  # (K, D)
    a1 = -sinv                                    # coeff of x^2 in logits
    a2 = 2.0 * mu_n * sinv                        # coeff of x
    c = (
        -np.sum(mu_n * mu_n * sinv, axis=1)
        + log_alpha.astype(np.float64)
        - 0.5 * np.sum(log_sigma.astype(np.float64), axis=1)
    )
    # center across K: softmax is invariant to per-n shifts, and this keeps
    # the on-device logits within exp()'s comfortable range (~±17)
    a1c = a1 - a1.mean(axis=0, keepdims=True)
    a2c = a2 - a2.mean(axis=0, keepdims=True)
    ccv = (c - c.mean()).astype(np.float32)
    w1 = np.ascontiguousarray(a1c.T, dtype=np.float16)  # (D, K)
    w2 = np.ascontiguousarray(a2c.T, dtype=np.float16)  # (D, K)
    cc = np.concatenate([ccv, ccv]).reshape(2 * K, 1)
    return w1, w2, cc


def _in_maps(x, mu, log_sigma, log_alpha):
    x16 = np.ascontiguousarray(np.asarray(x), dtype=np.float16)
    w1, w2, cc = _host_params(
        np.asarray(mu), np.asarray(log_sigma), np.asarray(log_alpha)
    )
    # float32 to match the F32R device declaration (f32r is an fp32 view)
    ones_bd = np.zeros((2 * K, 2 * K), dtype=np.float32)
    ones_bd[:K, :K] = 1
    ones_bd[K:, K:] = 1
    return [
        {
            "x": x16[i * BPC : (i + 1) * BPC],
            "w1": w1,
            "w2": w2,
            "cc": cc,
            "ones_bd": ones_bd,
        }
        for i in range(NCORES)
    ]


def kernel(x, mu, log_sigma, log_alpha):
    if "nc" not in _CACHE:
        _CACHE["nc"] = _build_nc()
    nc = _CACHE["nc"]
    in_maps = _in_maps(x, mu, log_sigma, log_alpha)
    res = run_bass_kernel_spmd(nc, in_maps, list(range(NCORES))).results
    out = np.concatenate(
        [np.asarray(res[i]["out"]) for i in range(NCORES)], axis=0
    )
    # unstack: dev[b, h*64+k, g*PW+c] = posterior[b, k, g*GRP + h*PW + c]
    n_grp = N // GRP
    out = (
        out.reshape(B, 2, K, n_grp, PW)
        .transpose(0, 2, 3, 1, 4)
        .reshape(B, K, N)
    )
    return out.astype(np.float32)


# revision 20
# speedup vs baseline: 1.1570x; 1.1570x over previous
"""Trainium2 Bass kernel for DiagonalGMMPosterior (vq_codebook).

Reference computation (per batch b, descriptor n, cluster k):
    dist[k,n]  = sum_d (x[d,n] - mu_n[k,d])^2 * exp(-log_sigma[k,d])
    logits     = -dist + log_alpha[k] - 0.5 * sum_d log_sigma[k,d]
    out[k,n]   = softmax_k(logits)

Device strategy (8 NeuronCores, data-parallel over the batch axis):
  * Host folds all (K,D) parameter math into two GEMM weight matrices and
    a per-cluster constant, then CENTERS them across K (softmax is
    invariant to per-n shifts) so logits stay within ~±17 — no per-n max.
  * x is pre-converted to fp16 on host: halves input HBM traffic.  fp16
    GEMMs also unlock PE column tiling (tile_position), which fp32r does
    not support.
  * Two 2048-column halves A/B of each 4096-column group are STACKED on
    the 128 PSUM partitions: A's dist GEMMs write partitions 0:64 (PE
    tile (0,0)), B's write 64:128 (tile (0,64)).  Everything downstream
    processes both halves in one 128-partition pass, halving the column
    count of exp / ones-GEMM / reciprocal / multiply:
       pd   = W1^T @ x^2 + W2^T @ x   stacked       (TensorE, fp16)
       et   = exp(pd + cc)                          (ScalarE -> f32r)
       pd   = ones_blockdiag^T @ et                 (TensorE overwrites
              the spent pd banks: per-half partition sum + broadcast;
              a separate pb pool would not fit the 8 PSUM banks)
       r    = 1/pd                                  (VectorE, ~18-bit)
       ot   = et * r -> fp16                        (split DVE / Pool)
  * Large groups (4096 cols) halve the cross-engine semaphore count —
    sync latency, not engine throughput, paced the smaller-tile version.
  * Elementwise work is split across DVE/ScalarE/Pool by measured HW
    rates (DVE ~0.65-1.1 ns/col, Act ~0.95 ns/col, Pool ~2.4 ns/col with
    ~1.2us fixed): square on DVE+Act, multiply on DVE+Pool.
  * fp16 output in the stacked layout; host unstacks and widens.
"""

import numpy as np

import concourse.bacc as bacc
import concourse.bass as bass
import concourse.tile as tile
from concourse import mybir
from concourse.bass_utils import run_bass_kernel_spmd

B, D, N, K = 16, 128, 16384, 64
NCORES = 8
BPC = B // NCORES   # batches per core
PW = 1024           # columns per stacked half (2 PSUM banks)
GRP = 2 * PW        # n-columns consumed per group (halves A+B)
SQ_DVE = 1024       # square columns on DVE; rest on ScalarE

F32 = mybir.dt.float32
F32R = mybir.dt.float32r
F16 = mybir.dt.float16

_CACHE = {}


def _build_nc():
    # Bacc (not raw Bass): its compile() pass legalizes Tile's multi-wait
    # instructions down to the 1-wait-per-instruction hardware limit.
    nc = bacc.Bacc("TRN2", target_bir_lowering=False, debug=False)
    x_in = nc.declare_dram_parameter("x", [BPC, D, N], F16, isOutput=False)
    w1_in = nc.declare_dram_parameter("w1", [D, K], F16, isOutput=False)
    w2_in = nc.declare_dram_parameter("w2", [D, K], F16, isOutput=False)
    cc_in = nc.declare_dram_parameter("cc", [2 * K, 1], F32, isOutput=False)
    ones_in = nc.declare_dram_parameter("ones_bd", [2 * K, 2 * K], F32R, isOutput=False)
    # output stays in the stacked layout; the host unstacks
    out_ext = nc.declare_dram_parameter("out", [BPC, 2 * K, N // 2], F16, isOutput=True)

    with tile.TileContext(nc) as tc:
        with (
            tc.tile_pool(name="consts", bufs=1) as consts,
            tc.tile_pool(name="xp", bufs=8) as xp,
            tc.tile_pool(name="xqp", bufs=3) as xqp,
            tc.tile_pool(name="ep", bufs=3) as ep,
            tc.tile_pool(name="rp", bufs=3) as rp,
            tc.tile_pool(name="op", bufs=2) as op,
            tc.tile_pool(name="pd", bufs=2, space="PSUM") as pdp,
            tc.tile_pool(name="pb", bufs=2, space="PSUM") as pbp,
        ):
            w1_sb = consts.tile([D, K], F16)
            nc.sync.dma_start(out=w1_sb, in_=w1_in[:, :])
            w2_sb = consts.tile([D, K], F16)
            nc.sync.dma_start(out=w2_sb, in_=w2_in[:, :])
            cc_sb = consts.tile([2 * K, 1], F32)
            nc.sync.dma_start(out=cc_sb, in_=cc_in[:, :])
            ones_bd = consts.tile([2 * K, 2 * K], F32R)
            nc.sync.dma_start(out=ones_bd, in_=ones_in[:, :])

            n_grp = N // GRP  # 4 per batch row
            groups = [(b, g) for b in range(BPC) for g in range(n_grp)]
            NG = len(groups)
            st = [dict() for _ in range(NG)]

            # software-pipelined emission: each engine's in-order stream
            # interleaves stages of consecutive groups so no stage
            # head-of-line-blocks the next group's earlier stage
            def s0_load(i):
                # one DMA covers two groups (16KB per partition row):
                # halves the per-dispatch sequencer cost (~590ns each)
                if i % 2:
                    st[i]["xt"] = st[i - 1]["xt_big"][:, GRP:]
                    return
                b, g = groups[i]
                n0 = g * GRP
                xt = xp.tile([D, 2 * GRP], F16, tag="xt")
                nc.sync.dma_start(out=xt, in_=x_in[b, :, n0 : n0 + 2 * GRP])
                st[i]["xt_big"] = xt
                st[i]["xt"] = xt[:, :GRP]

            def s1_square(i):
                xt = st[i]["xt"]
                xsq = xqp.tile([D, GRP], F16, tag="xsq")
                nc.vector.tensor_mul(
                    xsq[:, :SQ_DVE], xt[:, :SQ_DVE], xt[:, :SQ_DVE]
                )
                nc.scalar.activation(
                    out=xsq[:, SQ_DVE:], in_=xt[:, SQ_DVE:],
                    func=mybir.ActivationFunctionType.Square,
                )
                st[i]["xsq"] = xsq

            def s2_dist(i):
                xt, xsq = st[i]["xt"], st[i]["xsq"]
                # halves A (cols 0:PW) and B (cols PW:2PW) stacked on the
                # 128 PSUM partitions via PE column tiling.  Each PSUM
                # bank holds 512 fp32 columns -> 4 chains per half.
                pd_t = pdp.tile([2 * K, PW], F32, tag="pd")
                for half, p0 in ((0, 0), (1, K)):
                    coff = half * PW
                    for h in range(PW // 512):
                        sl = slice(h * 512, (h + 1) * 512)
                        msl = slice(coff + h * 512, coff + (h + 1) * 512)
                        nc.tensor.matmul(
                            pd_t[p0 : p0 + K, sl], w1_sb[:, :], xsq[:, msl],
                            start=True, stop=False,
                        )
                        nc.tensor.matmul(
                            pd_t[p0 : p0 + K, sl], w2_sb[:, :], xt[:, msl],
                            start=False, stop=True,
                        )
                st[i]["pd"] = pd_t

            def s3_exp(i):
                pd_t = st[i].pop("pd")
                # f32r: uniform-fp32 operands keep the DVE multiply on its
                # fast path (2-byte et forces a ~4x slower mixed-dtype
                # path) and stream the ones-GEMM at 1 cycle/row
                et = ep.tile([2 * K, PW], F32R, tag="et")
                nc.scalar.activation(
                    out=et, in_=pd_t,
                    func=mybir.ActivationFunctionType.Exp,
                    bias=cc_sb, scale=1.0,
                )
                st[i]["et"] = et
                st[i].pop("xt")
                st[i].pop("xsq")

            def s4_den(i):
                et = st[i]["et"]
                # block-diagonal ones: rows 0:64 sum half A, 64:128 sum
                # half B, each broadcast to its own partition range
                pb_t = pbp.tile([2 * K, PW], F32, tag="pb")
                for h in range(PW // 512):
                    sl = slice(h * 512, (h + 1) * 512)
                    nc.tensor.matmul(
                        pb_t[:, sl], ones_bd[:, :], et[:, sl],
                        start=True, stop=True,
                    )
                st[i]["pb"] = pb_t

            def s5_recip(i):
                pb_t = st[i].pop("pb")
                r_all = rp.tile([2 * K, PW], F32, tag="r")
                # ~18-bit-accurate custom-DVE reciprocal; the sum is
                # strictly positive and well inside fp32 normal range
                nc.vector.reciprocal_approx_fast(out=r_all, in_=pb_t)
                st[i]["r"] = r_all

            def s6_mult(i):
                et, r_all = st[i].pop("et"), st[i].pop("r")
                # double-wide output tile shared by group pairs: the store
                # then moves 8KB HBM rows in one dispatch per two groups
                if i % 2 == 0:
                    ot2 = op.tile([2 * K, 2 * PW], F16, tag="ot")
                    st[i]["ot2"] = ot2
                else:
                    ot2 = st[i - 1]["ot2"]
                c0 = (i % 2) * PW
                # the otherwise-idle Pool engine takes the whole multiply;
                # DVE keeps only square+reciprocal
                nc.gpsimd.tensor_mul(
                    ot2[:, c0 : c0 + PW], et.bitcast(F32), r_all
                )
                st[i]["ot"] = ot2

            def s7_store(i):
                if i % 2 == 0:
                    return
                b, g = groups[i]
                ot2 = st[i - 1].pop("ot2")
                st[i].pop("ot")
                st[i - 1].pop("ot")
                nc.sync.dma_start(
                    out=out_ext[b, :, (g - 1) * PW : (g + 1) * PW], in_=ot2[:, :]
                )

            for i in range(NG):
                s0_load(i)

            stages = [
                s1_square, s2_dist, s3_exp,
                s4_den, s5_recip, s6_mult, s7_store,
            ]
            NS = len(stages)
            # downstream stages emitted first within each tick so no
            # engine's in-order queue blocks a later group's earlier stage
            for tick in range(NG + NS - 1):
                for k in reversed(range(NS)):
                    i = tick - k
                    if 0 <= i < NG:
                        stages[k](i)
    nc.compile()
    return nc


def _host_params(mu, log_sigma, log_alpha):
    mu64 = mu.astype(np.float64)
    mu_n = mu64 / np.maximum(
        np.linalg.norm(mu64, axis=1, keepdims=True), 1e-12
    )
    sinv = np.exp(-log_sigma.astype(np.float64))  # (K, D)
    a1 = -sinv                                    # coeff of x^2 in logits
    a2 = 2.0 * mu_n * sinv                        # coeff of x
    c = (
        -np.sum(mu_n * mu_n * sinv, axis=1)
        + log_alpha.astype(np.float64)
        - 0.5 * np.sum(log_sigma.astype(np.float64), axis=1)
    )
    # center across K: softmax is invariant to per-n shifts, and this keeps
    # the on-device logits within exp()'s comfortable range (~±17)
    a1c = a1 - a1.mean(axis=0, keepdims=True)
    a2c = a2 - a2.mean(axis=0, keepdims=True)
    ccv = (c - c.mean()).astype(np.float32)
    w1 = np.ascontiguousarray(a1c.T, dtype=np.float16)  # (D, K)
    w2 = np.ascontiguousarray(a2c.T, dtype=np.float16)  # (D, K)
    cc = np.concatenate([ccv, ccv]).reshape(2 * K, 1)
    return w1, w2, cc


def _in_maps(x, mu, log_sigma, log_alpha):
    x16 = np.ascontiguousarray(np.asarray(x), dtype=np.float16)
    w1, w2, cc = _host_params(
        np.asarray(mu), np.asarray(log_sigma), np.asarray(log_alpha)
    )
    # float32 to match the F32R device declaration (f32r is an fp32 view)
    ones_bd = np.zeros((2 * K, 2 * K), dtype=np.float32)
    ones_bd[:K, :K] = 1
    ones_bd[K:, K:] = 1
    return [
        {
            "x": x16[i * BPC : (i + 1) * BPC],
            "w1": w1,
            "w2": w2,
            "cc": cc,
            "ones_bd": ones_bd,
        }
        for i in range(NCORES)
    ]


def kernel(x, mu, log_sigma, log_alpha):
    if "nc" not in _CACHE:
        _CACHE["nc"] = _build_nc()
    nc = _CACHE["nc"]
    in_maps = _in_maps(x, mu, log_sigma, log_alpha)
    res = run_bass_kernel_spmd(nc, in_maps, list(range(NCORES))).results
    out = np.concatenate(
        [np.asarray(res[i]["out"]) for i in range(NCORES)], axis=0
    )
    # unstack: dev[b, h*64+k, g*PW+c] = posterior[b, k, g*GRP + h*PW + c]
    n_grp = N // GRP
    out = (
        out.reshape(B, 2, K, n_grp, PW)
        .transpose(0, 2, 3, 1, 4)
        .reshape(B, K, N)
    )
    return out.astype(np.float32)
